# revision 15
# baseline (speedup 1.0000x reference)
"""Trainium2 Bass kernel for nn_BackBone_77532749627801 — stage 10.

Vs stage 2: all streamed weights and xT are pre-arranged host-side so every
DMA is a contiguous [128, N] block (SWDGE descriptor generation was a
hidden serializer for the rearrange-pattern loads); the V-bias broadcast is
hoisted out of the loop; the attn z/denominator tail batches 4 transposes
into one PSUM tile with a single vectored reciprocal (was a 24-hop
PE<->DVE ping-pong per pair); small PSUM users (K/Q eviction, V psum,
z-transpose) share a 3-slot rotation; and the AddNorm boundary offloads
elementwise work from DVE to the idle GPSIMD engine with transpose
evictions split across ACT and DVE.
"""

import contextlib
import os
import sys

import numpy as np

if "/opt/trn_rl_repo" not in sys.path and os.path.isdir("/opt/trn_rl_repo"):
    sys.path.insert(0, "/opt/trn_rl_repo")

B, S, D, H, DH, F = 4, 2048, 1024, 16, 64, 4096
N_CORES = 8
TOK = 1024  # query tokens per core
EPS = 1e-5
EXP_SHIFT = -3.0  # constant shift inside exp; cancels in softmax
XS = 8.0     # fp8 scale on x
WS = 256.0   # fp8 scale on Wq/Wk/Wv
PS = XS * WS  # scale of fp8 projection PSUM results

_BUILD_CACHE = {}


def _build(n_iters=1):
    import concourse.bacc as bacc
    import concourse.mybir as mybir
    import concourse.tile as tile
    from concourse.masks import make_identity
    from contextlib import ExitStack

    f32 = mybir.dt.float32
    bf16 = mybir.dt.bfloat16
    fp8 = mybir.dt.float8e4
    DR = mybir.MatmulPerfMode.DoubleRow
    AF = mybir.ActivationFunctionType

    nc = bacc.Bacc("TRN2", target_bir_lowering=False, debug=False,
                   num_devices=N_CORES)

    # host-packed layouts: every DMA below is a contiguous [128, N] block
    xT = nc.dram_tensor("xT", [128, 8, S], fp8, kind="ExternalInput").ap()
    xh = nc.dram_tensor("xh", [TOK, D], f32, kind="ExternalInput").ap()
    wq = nc.dram_tensor("wq", [8, 128, 8, 128], fp8, kind="ExternalInput").ap()
    wk = nc.dram_tensor("wk", [8, 128, 8, 128], fp8, kind="ExternalInput").ap()
    wv = nc.dram_tensor("wv", [4, 128, 8, 260], fp8, kind="ExternalInput").ap()
    bqk = nc.dram_tensor("bqk", [2, D], f32, kind="ExternalInput").ap()
    bv4 = nc.dram_tensor("bv4", [1, 4, 260], f32, kind="ExternalInput").ap()
    w1 = nc.dram_tensor("w1", [32, 128, 8, 128], bf16,
                        kind="ExternalInput").ap()
    b1d = nc.dram_tensor("b1", [F], f32, kind="ExternalInput").ap()
    w2 = nc.dram_tensor("w2", [F, D], bf16, kind="ExternalInput").ap()
    b2r = nc.dram_tensor("b2r", [1, D], f32, kind="ExternalInput").ap()
    ln2g = nc.dram_tensor("ln2g", [D], f32, kind="ExternalInput").ap()
    ln2b = nc.dram_tensor("ln2b", [D], f32, kind="ExternalInput").ap()
    ln1g = nc.dram_tensor("ln1g", [D], f32, kind="ExternalInput").ap()
    out = nc.dram_tensor("out", [TOK, D], f32, kind="ExternalOutput").ap()

    with tile.TileContext(nc) as tc, ExitStack() as top:
        const = top.enter_context(tc.tile_pool(name="const", bufs=1))
        ident_f = const.tile([128, 128], f32)
        make_identity(nc, ident_f)
        eshift = const.tile([128, 1], f32)
        nc.vector.memset(eshift, EXP_SHIFT)
        eps_t = const.tile([128, 1], f32)
        nc.vector.memset(eps_t, EPS)
        bq_sb = const.tile([128, 8], f32)
        nc.scalar.dma_start(out=bq_sb, in_=bqk[0].rearrange("(pr p) -> p pr", p=128))
        bk_sb = const.tile([128, 8], f32)
        nc.scalar.dma_start(out=bk_sb, in_=bqk[1].rearrange("(pr p) -> p pr", p=128))
        b1_sb = const.tile([128, 32], f32)
        nc.scalar.dma_start(out=b1_sb, in_=b1d.rearrange("(fc p) -> p fc", p=128))
        g1_bc = const.tile([128, D], f32)
        nc.gpsimd.dma_start(out=g1_bc, in_=ln1g.partition_broadcast(128))
        bv_bc = const.tile([128, 4, 260], f32)
        for quad in range(4):
            nc.gpsimd.dma_start(
                out=bv_bc[:, quad, :],
                in_=bv4[0, quad, :].partition_broadcast(128))

        resid = top.enter_context(tc.tile_pool(name="resid", bufs=1))

        loop = tc.For_i(0, n_iters) if n_iters > 1 else contextlib.nullcontext()
        with loop:
            mha = resid.tile([128, 8, D], f32, tag="mha")
            xh_sb = resid.tile([128, 8, D], f32, tag="xh")
            for st in range(8):
                nc.gpsimd.dma_start(
                    out=xh_sb[:, st, :],
                    in_=xh[st * 128:(st + 1) * 128, :])

            # ---------------- Phase A: QKV + attention ----------------
            with ExitStack() as pha:
                xpool = pha.enter_context(tc.tile_pool(name="xT", bufs=1))
                wp = pha.enter_context(tc.tile_pool(name="wpair", bufs=3))
                att = pha.enter_context(tc.tile_pool(name="att", bufs=1))
                zpool = pha.enter_context(tc.tile_pool(name="zp", bufs=3))
                psA = pha.enter_context(
                    tc.tile_pool(name="psA", bufs=1, space="PSUM"))

                xT_sb = xpool.tile([128, 8, S], fp8)
                for dt in range(8):
                    nc.sync.dma_start(out=xT_sb[:, dt, :], in_=xT[:, dt, :])

                for quad in range(4):
                    # V + bias (+ ones cols) for 4 heads: vplus[t, 65j+e]
                    wv_sb = wp.tile([128, 8, 260], fp8, tag="wv")
                    nc.gpsimd.dma_start(out=wv_sb, in_=wv[quad])
                    vplus = att.tile([128, 16, 260], bf16, tag="vplus", bufs=2)
                    for tt in range(16):
                        vp_ps = psA.tile([128, 260], f32, tag="small", bufs=1)
                        for dr in range(4):
                            nc.tensor.matmul(
                                out=vp_ps,
                                lhsT=xT_sb[:, 2 * dr:2 * dr + 2,
                                           tt * 128:(tt + 1) * 128],
                                rhs=wv_sb[:, 2 * dr:2 * dr + 2, :],
                                perf_mode=DR,
                                start=(dr == 0), stop=(dr == 3))
                        nc.vector.scalar_tensor_tensor(
                            out=vplus[:, tt, :], in0=vp_ps, scalar=1.0 / PS,
                            in1=bv_bc[:, quad, :],
                            op0=mybir.AluOpType.mult,
                            op1=mybir.AluOpType.add)

                    for pr01 in range(2):
                        pair = 2 * quad + pr01
                        wk_sb = wp.tile([128, 8, 128], fp8, tag="wk")
                        nc.gpsimd.dma_start(out=wk_sb, in_=wk[pair])
                        wq_sb = wp.tile([128, 8, 128], fp8, tag="wq")
                        nc.gpsimd.dma_start(out=wq_sb, in_=wq[pair])

                        # kT/qT hold PS * (K + bk): biases pre-scaled host-side
                        kT = att.tile([128, S], bf16, tag="kT", bufs=2)
                        for ch in range(4):
                            kq_ps = psA.tile([128, 512], f32, tag="kq",
                                             bufs=2, name=f"kq_{pair}_{ch}")
                            for dr in range(4):
                                nc.tensor.matmul(
                                    out=kq_ps,
                                    lhsT=wk_sb[:, 2 * dr:2 * dr + 2, :],
                                    rhs=xT_sb[:, 2 * dr:2 * dr + 2,
                                              ch * 512:(ch + 1) * 512],
                                    perf_mode=DR,
                                    start=(dr == 0), stop=(dr == 3))
                            nc.vector.tensor_scalar_add(
                                out=kT[:, ch * 512:(ch + 1) * 512],
                                in0=kq_ps, scalar1=bk_sb[:, pair:pair + 1])

                        # own tokens are xT columns 0:1024 (host rotation)
                        qT = att.tile([128, TOK], bf16, tag="qT", bufs=2)
                        for ch in range(2):
                            kq_ps = psA.tile([128, 512], f32, tag="kq",
                                             bufs=2, name=f"qq_{pair}_{ch}")
                            for dr in range(4):
                                nc.tensor.matmul(
                                    out=kq_ps,
                                    lhsT=wq_sb[:, 2 * dr:2 * dr + 2, :],
                                    rhs=xT_sb[:, 2 * dr:2 * dr + 2,
                                              ch * 512:(ch + 1) * 512],
                                    perf_mode=DR,
                                    start=(dr == 0), stop=(dr == 3))
                            nc.vector.tensor_scalar_add(
                                out=qT[:, ch * 512:(ch + 1) * 512],
                                in0=kq_ps, scalar1=bq_sb[:, pair:pair + 1])

                        # scores + softmax-exp + attn@V for the two heads of
                        # this pair.  K=64 scores matmuls run row-packed; one
                        # [128, 2048] PSUM tile covers both heads so each exp
                        # instruction is N=2048.
                        for sch in range(2):
                            expT = [att.tile([128, 16, 512], bf16,
                                             tag=f"expT{h01}", bufs=2,
                                             name=f"expT{h01}_{pair}_{sch}")
                                    for h01 in range(2)]
                            for tp in range(8):
                                sT_ps = [psA.tile([128, 1024], f32,
                                                  tag="sT", bufs=2,
                                                  name=f"sT_{h01}_{tp}")
                                         for h01 in range(2)]
                                for sub in range(2):
                                    tt = 2 * tp + sub
                                    for h01 in range(2):
                                        pslice = slice(h01 * 64, h01 * 64 + 64)
                                        nc.tensor.matmul(
                                            out=sT_ps[h01][:, sub * 512:
                                                           sub * 512 + 512],
                                            lhsT=kT[pslice,
                                                    tt * 128:(tt + 1) * 128],
                                            rhs=qT[pslice,
                                                   sch * 512:(sch + 1) * 512],
                                            start=True, stop=True)
                                for h01 in range(2):
                                    nc.scalar.activation(
                                        out=expT[h01][:, 2 * tp:2 * tp + 2, :],
                                        in_=sT_ps[h01], func=AF.Exp,
                                        bias=eshift[:, :],
                                        scale=0.125 / (PS * PS))
                            for h01 in range(2):
                                head = 2 * pair + h01
                                j = 2 * pr01 + h01
                                zT_ps = psA.tile([65, 512], f32, tag="zT",
                                                 bufs=1)
                                for tt in range(16):
                                    nc.tensor.matmul(
                                        out=zT_ps,
                                        lhsT=vplus[:, tt, 65 * j:65 * j + 65],
                                        rhs=expT[h01][:, tt, :],
                                        start=(tt == 0), stop=(tt == 15))
                                zT_sb = zpool.tile([65, 512], f32, tag="zT_sb")
                                nc.vector.tensor_copy(out=zT_sb, in_=zT_ps)
                                ztr4 = psA.tile([128, 4, 65], f32,
                                                tag="small", bufs=1,
                                                name=f"ztr_{head}_{sch}")
                                for sb4 in range(4):
                                    nc.tensor.transpose(
                                        out=ztr4[:, sb4, :],
                                        in_=zT_sb[:, sb4 * 128:(sb4 + 1) * 128],
                                        identity=ident_f[0:65, 0:65])
                                rec4 = zpool.tile([128, 4, 1], f32, tag="rec")
                                nc.vector.reciprocal(
                                    out=rec4, in_=ztr4[:, :, 64:65])
                                nc.vector.tensor_mul(
                                    out=mha[:, sch * 4:sch * 4 + 4,
                                            head * 64:head * 64 + 64],
                                    in0=ztr4[:, :, 0:64],
                                    in1=rec4.broadcast_to([128, 4, 64]))

            # ---------------- Phase B: AddNorm1 + FFN + AddNorm2 --------
            with ExitStack() as phb:
                bpool = phb.enter_context(tc.tile_pool(name="bpool", bufs=1))
                stream = phb.enter_context(tc.tile_pool(name="stream", bufs=4))
                stat = phb.enter_context(tc.tile_pool(name="stat", bufs=4))
                g2_bc = bpool.tile([128, D], f32, tag="g2")
                nc.gpsimd.dma_start(out=g2_bc,
                                    in_=ln2g.partition_broadcast(128))
                b2_bc = bpool.tile([128, D], f32, tag="b2")
                nc.gpsimd.dma_start(out=b2_bc,
                                    in_=ln2b.partition_broadcast(128))
                ffb2_bc = bpool.tile([128, D], f32, tag="ffb2")
                nc.gpsimd.dma_start(out=ffb2_bc,
                                    in_=b2r[0, :].partition_broadcast(128))

                def layer_norm_inplace(g_bc, b_bc, add_in=None,
                                       out_dma=False):
                    for st in range(8):
                        h = mha[:, st, :]
                        if add_in is not None:
                            nc.vector.tensor_add(out=h, in0=h,
                                                 in1=add_in[:, st, :])
                        stats = stat.tile([128, 2, 6], f32, tag="stats")
                        for sg in range(2):
                            nc.vector.bn_stats(
                                out=stats[:, sg, :],
                                in_=h[:, sg * 512:(sg + 1) * 512])
                        mv = stat.tile([128, 2], f32, tag="mv")
                        nc.vector.bn_aggr(out=mv, in_=stats)
                        nc.scalar.activation(
                            out=mv[:, 1:2], in_=mv[:, 1:2],
                            func=AF.Sqrt, bias=eps_t[:, :])
                        nc.vector.reciprocal(out=mv[:, 1:2], in_=mv[:, 1:2])
                        nc.vector.tensor_scalar(
                            out=h, in0=h, scalar1=mv[:, 0:1],
                            scalar2=mv[:, 1:2],
                            op0=mybir.AluOpType.subtract,
                            op1=mybir.AluOpType.mult)
                        if g_bc is not None:
                            nc.vector.tensor_mul(out=h, in0=h, in1=g_bc[:, :])
                            nc.vector.tensor_add(out=h, in0=h, in1=b_bc[:, :])
                        if out_dma:
                            nc.sync.dma_start(
                                out=out.rearrange(
                                    "(st p) d -> p st d", p=128)[:, st, :],
                                in_=h)

                # LN1 gamma/beta are folded host-side into W1/b1 (FFN path)
                # and into b2r (residual path); apply only the normalize here
                # so h1T/FFN1 can start sooner.  mha holds y = normalized.
                layer_norm_inplace(None, None, add_in=xh_sb)

                # h1T[d, s] in bf16 (FFN1 moving operand); PSUM evictions
                # split between ACT and DVE
                h1T = bpool.tile([128, 8, TOK], bf16, tag="h1T")
                with tc.tile_pool(name="psT", bufs=1, space="PSUM") as psT:
                    for st in range(8):
                        for dt in range(8):
                            tr_ps = psT.tile([128, 128], f32, tag="tr", bufs=4)
                            nc.tensor.transpose(
                                out=tr_ps,
                                in_=mha[:, st, dt * 128:(dt + 1) * 128],
                                identity=ident_f[:, :])
                            nc.vector.tensor_copy(
                                out=h1T[:, dt, st * 128:(st + 1) * 128],
                                in_=tr_ps)

                # residual stream: h1 = y*g1 + (ln1_b folded into b2r)
                for st in range(8):
                    nc.vector.tensor_mul(out=mha[:, st, :], in0=mha[:, st, :],
                                         in1=g1_bc[:, :])

                # FFN pass 1: aT[Fc] = gelu(W1^T h1 + b1) -> SBUF (bf16)
                aT_all = bpool.tile([128, 32, TOK], bf16, tag="aT")
                with tc.tile_pool(name="ps1", bufs=1, space="PSUM") as ps1:
                    for fc in range(32):
                        w1t = stream.tile([128, 8, 128], bf16, tag="w1t", bufs=6)
                        nc.sync.dma_start(out=w1t, in_=w1[fc])
                        a_ps = ps1.tile([128, TOK], f32, tag="aps", bufs=3)
                        for nh in range(2):
                            for dt in range(8):
                                nc.tensor.matmul(
                                    out=a_ps[:, nh * 512:(nh + 1) * 512],
                                    lhsT=w1t[:, dt, :],
                                    rhs=h1T[:, dt, nh * 512:(nh + 1) * 512],
                                    start=(dt == 0), stop=(dt == 7))
                        nc.scalar.activation(
                            out=aT_all[:, fc, :], in_=a_ps, func=AF.Gelu,
                            bias=b1_sb[:, fc:fc + 1])

                # FFN pass 2: ff = aT^T @ W2 + b2; z2 = h1 + ff (into mha)
                with tc.tile_pool(name="ps2", bufs=1, space="PSUM") as ps2:
                    for sh in range(2):
                        ffps = [ps2.tile([128, D], f32, tag="ff", bufs=4,
                                         name=f"ff_{sh}_{i}")
                                for i in range(4)]
                        for fc in range(32):
                            w2t = stream.tile([128, D], bf16, tag="w2t", bufs=6)
                            nc.scalar.dma_start(
                                out=w2t, in_=w2[fc * 128:(fc + 1) * 128, :])
                            for st2 in range(4):
                                for nh in range(2):
                                    nc.tensor.matmul(
                                        out=ffps[st2][:, nh * 512:(nh + 1) * 512],
                                        lhsT=aT_all[:, fc,
                                                    (sh * 4 + st2) * 128:
                                                    (sh * 4 + st2 + 1) * 128],
                                        rhs=w2t[:, nh * 512:(nh + 1) * 512],
                                        start=(fc == 0), stop=(fc == 31))
                        for st2 in range(4):
                            stg = sh * 4 + st2
                            nc.vector.tensor_add(
                                out=mha[:, stg, :], in0=mha[:, stg, :],
                                in1=ffps[st2][:, :])
                            nc.vector.tensor_add(
                                out=mha[:, stg, :], in0=mha[:, stg, :],
                                in1=ffb2_bc[:, :])

                layer_norm_inplace(g2_bc, b2_bc, out_dma=True)

    nc.compile()
    return nc


def _pack_inputs(x, Wq, bq, Wk, bk, Wv, bv, ln1_g, ln1_b, W1, b1, W2, b2,
                 ln2_g, ln2_b):
    """Build the 8 per-core input maps (host-side, numpy)."""
    import ml_dtypes
    import concourse.mybir as mybir

    f = np.float32
    bf = ml_dtypes.bfloat16
    f8 = mybir.dt.np(mybir.dt.float8e4)

    def pack_w(arr, groups, m):
        # [D, groups*m] -> [groups, 128, 8, m] with [g][p][dt][:] contiguous
        return np.ascontiguousarray(
            arr.reshape(8, 128, groups, m).transpose(2, 1, 0, 3))

    wq_all = (np.ascontiguousarray(
        np.transpose(np.asarray(Wq, f), (1, 0, 2)).reshape(D, D)) * WS
    ).astype(f8)
    wk_all = (np.ascontiguousarray(
        np.transpose(np.asarray(Wk, f), (1, 0, 2)).reshape(D, D)) * WS
    ).astype(f8)
    Wv_ = np.asarray(Wv, f)
    bv_ = np.asarray(bv, f)
    wv_all = np.zeros((D, 4, 260), f)
    bv_all = np.zeros((1, 4, 260), f)
    for quad in range(4):
        for j in range(4):
            h = quad * 4 + j
            wv_all[:, quad, 65 * j:65 * j + 64] = Wv_[h]
            bv_all[0, quad, 65 * j:65 * j + 64] = bv_[h]
            bv_all[0, quad, 65 * j + 64] = 1.0
    wv_p = np.ascontiguousarray(
        ((wv_all * WS).astype(f8)).reshape(8, 128, 4, 260).transpose(2, 1, 0, 3))
    # QK biases pre-scaled so fp8-scaled PSUM + bias stays on one scale
    bqk = np.stack([np.asarray(bq, f).reshape(D) * PS,
                    np.asarray(bk, f).reshape(D) * PS])
    x = np.asarray(x, f)
    W1_ = np.asarray(W1, np.float64)
    g1_ = np.asarray(ln1_g, np.float64)
    bb1_ = np.asarray(ln1_b, np.float64)
    w1_folded = (g1_[:, None] * W1_).astype(f)
    b1_folded = (np.asarray(b1, np.float64) + bb1_ @ W1_).astype(f)
    b2_folded = (np.asarray(b2, np.float64) + bb1_).astype(f)
    common = dict(
        wq=pack_w(wq_all, 8, 128), wk=pack_w(wk_all, 8, 128), wv=wv_p,
        bqk=bqk, bv4=bv_all,
        w1=pack_w(w1_folded.astype(bf), 32, 128), b1=b1_folded,
        w2=np.asarray(W2, f).astype(bf),
        b2r=b2_folded.reshape(1, D),
        ln1g=np.asarray(ln1_g, f),
        ln2g=np.asarray(ln2_g, f), ln2b=np.asarray(ln2_b, f))
    in_maps = []
    for c in range(N_CORES):
        b_, half = c // 2, c % 2
        m = dict(common)
        own = x[b_, half * TOK:(half + 1) * TOK]
        other = x[b_, (1 - half) * TOK:(2 - half) * TOK]
        xTc = (np.ascontiguousarray(
            np.concatenate([own, other], axis=0).T) * XS).astype(f8)
        m["xT"] = np.ascontiguousarray(
            xTc.reshape(8, 128, S).transpose(1, 0, 2))
        m["xh"] = np.ascontiguousarray(own)
        in_maps.append(m)
    return in_maps


def kernel(**inputs):
    from concourse.bass_utils import run_bass_kernel_spmd

    if "nc" not in _BUILD_CACHE:
        _BUILD_CACHE["nc"] = _build()
    nc = _BUILD_CACHE["nc"]
    in_maps = _pack_inputs(**inputs)
    res = run_bass_kernel_spmd(nc, in_maps, core_ids=list(range(N_CORES)))
    out = np.zeros((B, S, D), np.float32)
    for c in range(N_CORES):
        b_, half = c // 2, c % 2
        out[b_, half * TOK:(half + 1) * TOK] = res.results[c]["out"]
    return out


# revision 16
# speedup vs baseline: 1.0448x; 1.0448x over previous
"""Trainium2 Bass kernel for nn_BackBone_77532749627801 — stage 7.

Vs stage 2: all streamed weights and xT are pre-arranged host-side so every
DMA is a contiguous [128, N] block (SWDGE descriptor generation was a
hidden serializer for the rearrange-pattern loads); the V-bias broadcast is
hoisted out of the loop; the attn z/denominator tail batches 4 transposes
into one PSUM tile with a single vectored reciprocal (was a 24-hop
PE<->DVE ping-pong per pair); small PSUM users (K/Q eviction, V psum,
z-transpose) share a 3-slot rotation; and the AddNorm boundary offloads
elementwise work from DVE to the idle GPSIMD engine with transpose
evictions split across ACT and DVE.
"""

import contextlib
import os
import sys

import numpy as np

if "/opt/trn_rl_repo" not in sys.path and os.path.isdir("/opt/trn_rl_repo"):
    sys.path.insert(0, "/opt/trn_rl_repo")

B, S, D, H, DH, F = 4, 2048, 1024, 16, 64, 4096
N_CORES = 8
TOK = 1024  # query tokens per core
EPS = 1e-5
EXP_SHIFT = -3.0  # constant shift inside exp; cancels in softmax
XS = 8.0     # fp8 scale on x
WS = 256.0   # fp8 scale on Wq/Wk/Wv
PS = XS * WS  # scale of fp8 projection PSUM results

_BUILD_CACHE = {}


def _build(n_iters=1):
    import concourse.bacc as bacc
    import concourse.mybir as mybir
    import concourse.tile as tile
    from concourse.masks import make_identity
    from contextlib import ExitStack

    f32 = mybir.dt.float32
    bf16 = mybir.dt.bfloat16
    fp8 = mybir.dt.float8e4
    DR = mybir.MatmulPerfMode.DoubleRow
    AF = mybir.ActivationFunctionType

    nc = bacc.Bacc("TRN2", target_bir_lowering=False, debug=False,
                   num_devices=N_CORES)

    # host-packed layouts: every DMA below is a contiguous [128, N] block
    xT = nc.dram_tensor("xT", [128, 8, S], fp8, kind="ExternalInput").ap()
    xh = nc.dram_tensor("xh", [TOK, D], f32, kind="ExternalInput").ap()
    wq = nc.dram_tensor("wq", [8, 128, 8, 128], fp8, kind="ExternalInput").ap()
    wk = nc.dram_tensor("wk", [8, 128, 8, 128], fp8, kind="ExternalInput").ap()
    wv = nc.dram_tensor("wv", [4, 128, 8, 260], fp8, kind="ExternalInput").ap()
    bqk = nc.dram_tensor("bqk", [2, D], f32, kind="ExternalInput").ap()
    bv4 = nc.dram_tensor("bv4", [1, 4, 260], f32, kind="ExternalInput").ap()
    w1 = nc.dram_tensor("w1", [32, 128, 8, 128], bf16,
                        kind="ExternalInput").ap()
    b1d = nc.dram_tensor("b1", [F], f32, kind="ExternalInput").ap()
    w2 = nc.dram_tensor("w2", [F, D], bf16, kind="ExternalInput").ap()
    b2r = nc.dram_tensor("b2r", [1, D], f32, kind="ExternalInput").ap()
    ln2g = nc.dram_tensor("ln2g", [D], f32, kind="ExternalInput").ap()
    ln2b = nc.dram_tensor("ln2b", [D], f32, kind="ExternalInput").ap()
    ln1g = nc.dram_tensor("ln1g", [D], f32, kind="ExternalInput").ap()
    out = nc.dram_tensor("out", [TOK, D], f32, kind="ExternalOutput").ap()

    with tile.TileContext(nc) as tc, ExitStack() as top:
        const = top.enter_context(tc.tile_pool(name="const", bufs=1))
        ident_f = const.tile([128, 128], f32)
        make_identity(nc, ident_f)
        eshift = const.tile([128, 1], f32)
        nc.vector.memset(eshift, EXP_SHIFT)
        eps_t = const.tile([128, 1], f32)
        nc.vector.memset(eps_t, EPS)
        bq_sb = const.tile([128, 8], f32)
        nc.scalar.dma_start(out=bq_sb, in_=bqk[0].rearrange("(pr p) -> p pr", p=128))
        bk_sb = const.tile([128, 8], f32)
        nc.scalar.dma_start(out=bk_sb, in_=bqk[1].rearrange("(pr p) -> p pr", p=128))
        b1_sb = const.tile([128, 32], f32)
        nc.scalar.dma_start(out=b1_sb, in_=b1d.rearrange("(fc p) -> p fc", p=128))
        g1_bc = const.tile([128, D], f32)
        nc.gpsimd.dma_start(out=g1_bc, in_=ln1g.partition_broadcast(128))
        bv_bc = const.tile([128, 4, 260], f32)
        for quad in range(4):
            nc.gpsimd.dma_start(
                out=bv_bc[:, quad, :],
                in_=bv4[0, quad, :].partition_broadcast(128))

        resid = top.enter_context(tc.tile_pool(name="resid", bufs=1))

        loop = tc.For_i(0, n_iters) if n_iters > 1 else contextlib.nullcontext()
        with loop:
            mha = resid.tile([128, 8, D], f32, tag="mha")
            xh_sb = resid.tile([128, 8, D], f32, tag="xh")
            for st in range(8):
                nc.gpsimd.dma_start(
                    out=xh_sb[:, st, :],
                    in_=xh[st * 128:(st + 1) * 128, :])

            # ---------------- Phase A: QKV + attention ----------------
            with ExitStack() as pha:
                xpool = pha.enter_context(tc.tile_pool(name="xT", bufs=1))
                wp = pha.enter_context(tc.tile_pool(name="wpair", bufs=3))
                att = pha.enter_context(tc.tile_pool(name="att", bufs=1))
                zpool = pha.enter_context(tc.tile_pool(name="zp", bufs=2))
                psA = pha.enter_context(
                    tc.tile_pool(name="psA", bufs=1, space="PSUM"))

                xT_sb = xpool.tile([128, 8, S], fp8)
                for dt in range(8):
                    nc.sync.dma_start(out=xT_sb[:, dt, :], in_=xT[:, dt, :])

                for quad in range(4):
                    # V + bias (+ ones cols) for 4 heads: vplus[t, 65j+e]
                    wv_sb = wp.tile([128, 8, 260], fp8, tag="wv")
                    nc.sync.dma_start(out=wv_sb, in_=wv[quad])
                    vplus = att.tile([128, 16, 260], bf16, tag="vplus", bufs=2)
                    for tt in range(16):
                        vp_ps = psA.tile([128, 260], f32, tag="small", bufs=1)
                        for dr in range(4):
                            nc.tensor.matmul(
                                out=vp_ps,
                                lhsT=xT_sb[:, 2 * dr:2 * dr + 2,
                                           tt * 128:(tt + 1) * 128],
                                rhs=wv_sb[:, 2 * dr:2 * dr + 2, :],
                                perf_mode=DR,
                                start=(dr == 0), stop=(dr == 3))
                        nc.vector.scalar_tensor_tensor(
                            out=vplus[:, tt, :], in0=vp_ps, scalar=1.0 / PS,
                            in1=bv_bc[:, quad, :],
                            op0=mybir.AluOpType.mult,
                            op1=mybir.AluOpType.add)

                    for pr01 in range(2):
                        pair = 2 * quad + pr01
                        wk_sb = wp.tile([128, 8, 128], fp8, tag="wk")
                        nc.sync.dma_start(out=wk_sb, in_=wk[pair])
                        wq_sb = wp.tile([128, 8, 128], fp8, tag="wq")
                        nc.sync.dma_start(out=wq_sb, in_=wq[pair])

                        # kT/qT hold PS * (K + bk): biases pre-scaled host-side
                        kT = att.tile([128, S], bf16, tag="kT", bufs=2)
                        for ch in range(4):
                            kq_ps = psA.tile([128, 512], f32, tag="kq",
                                             bufs=2, name=f"kq_{pair}_{ch}")
                            for dr in range(4):
                                nc.tensor.matmul(
                                    out=kq_ps,
                                    lhsT=wk_sb[:, 2 * dr:2 * dr + 2, :],
                                    rhs=xT_sb[:, 2 * dr:2 * dr + 2,
                                              ch * 512:(ch + 1) * 512],
                                    perf_mode=DR,
                                    start=(dr == 0), stop=(dr == 3))
                            nc.vector.tensor_scalar_add(
                                out=kT[:, ch * 512:(ch + 1) * 512],
                                in0=kq_ps, scalar1=bk_sb[:, pair:pair + 1])

                        # own tokens are xT columns 0:1024 (host rotation)
                        qT = att.tile([128, TOK], bf16, tag="qT", bufs=2)
                        for ch in range(2):
                            kq_ps = psA.tile([128, 512], f32, tag="kq",
                                             bufs=2, name=f"qq_{pair}_{ch}")
                            for dr in range(4):
                                nc.tensor.matmul(
                                    out=kq_ps,
                                    lhsT=wq_sb[:, 2 * dr:2 * dr + 2, :],
                                    rhs=xT_sb[:, 2 * dr:2 * dr + 2,
                                              ch * 512:(ch + 1) * 512],
                                    perf_mode=DR,
                                    start=(dr == 0), stop=(dr == 3))
                            nc.vector.tensor_scalar_add(
                                out=qT[:, ch * 512:(ch + 1) * 512],
                                in0=kq_ps, scalar1=bq_sb[:, pair:pair + 1])

                        # scores + softmax-exp + attn@V for the two heads of
                        # this pair.  K=64 scores matmuls run row-packed; one
                        # [128, 2048] PSUM tile covers both heads so each exp
                        # instruction is N=2048.
                        for sch in range(2):
                            expT = [att.tile([128, 16, 512], bf16,
                                             tag=f"expT{h01}", bufs=2,
                                             name=f"expT{h01}_{pair}_{sch}")
                                    for h01 in range(2)]
                            for tp in range(8):
                                sT_ps = [psA.tile([128, 1024], f32,
                                                  tag="sT", bufs=2,
                                                  name=f"sT_{h01}_{tp}")
                                         for h01 in range(2)]
                                for sub in range(2):
                                    tt = 2 * tp + sub
                                    for h01 in range(2):
                                        pslice = slice(h01 * 64, h01 * 64 + 64)
                                        nc.tensor.matmul(
                                            out=sT_ps[h01][:, sub * 512:
                                                           sub * 512 + 512],
                                            lhsT=kT[pslice,
                                                    tt * 128:(tt + 1) * 128],
                                            rhs=qT[pslice,
                                                   sch * 512:(sch + 1) * 512],
                                            start=True, stop=True)
                                for h01 in range(2):
                                    nc.scalar.activation(
                                        out=expT[h01][:, 2 * tp:2 * tp + 2, :],
                                        in_=sT_ps[h01], func=AF.Exp,
                                        bias=eshift[:, :],
                                        scale=0.125 / (PS * PS))
                            for h01 in range(2):
                                head = 2 * pair + h01
                                j = 2 * pr01 + h01
                                zT_ps = psA.tile([65, 512], f32, tag="zT",
                                                 bufs=1)
                                for tt in range(16):
                                    nc.tensor.matmul(
                                        out=zT_ps,
                                        lhsT=vplus[:, tt, 65 * j:65 * j + 65],
                                        rhs=expT[h01][:, tt, :],
                                        start=(tt == 0), stop=(tt == 15))
                                zT_sb = zpool.tile([65, 512], f32, tag="zT_sb")
                                nc.vector.tensor_copy(out=zT_sb, in_=zT_ps)
                                ztr4 = psA.tile([128, 4, 65], f32,
                                                tag="small", bufs=1,
                                                name=f"ztr_{head}_{sch}")
                                for sb4 in range(4):
                                    nc.tensor.transpose(
                                        out=ztr4[:, sb4, :],
                                        in_=zT_sb[:, sb4 * 128:(sb4 + 1) * 128],
                                        identity=ident_f[0:65, 0:65])
                                rec4 = zpool.tile([128, 4, 1], f32, tag="rec")
                                nc.vector.reciprocal(
                                    out=rec4, in_=ztr4[:, :, 64:65])
                                nc.vector.tensor_mul(
                                    out=mha[:, sch * 4:sch * 4 + 4,
                                            head * 64:head * 64 + 64],
                                    in0=ztr4[:, :, 0:64],
                                    in1=rec4.broadcast_to([128, 4, 64]))

            # ---------------- Phase B: AddNorm1 + FFN + AddNorm2 --------
            with ExitStack() as phb:
                bpool = phb.enter_context(tc.tile_pool(name="bpool", bufs=1))
                stream = phb.enter_context(tc.tile_pool(name="stream", bufs=4))
                stat = phb.enter_context(tc.tile_pool(name="stat", bufs=4))
                g2_bc = bpool.tile([128, D], f32, tag="g2")
                nc.gpsimd.dma_start(out=g2_bc,
                                    in_=ln2g.partition_broadcast(128))
                b2_bc = bpool.tile([128, D], f32, tag="b2")
                nc.gpsimd.dma_start(out=b2_bc,
                                    in_=ln2b.partition_broadcast(128))
                ffb2_bc = bpool.tile([128, D], f32, tag="ffb2")
                nc.gpsimd.dma_start(out=ffb2_bc,
                                    in_=b2r[0, :].partition_broadcast(128))

                def layer_norm_inplace(g_bc, b_bc, add_in=None,
                                       out_dma=False):
                    for st in range(8):
                        h = mha[:, st, :]
                        if add_in is not None:
                            nc.vector.tensor_add(out=h, in0=h,
                                                 in1=add_in[:, st, :])
                        stats = stat.tile([128, 2, 6], f32, tag="stats")
                        for sg in range(2):
                            nc.vector.bn_stats(
                                out=stats[:, sg, :],
                                in_=h[:, sg * 512:(sg + 1) * 512])
                        mv = stat.tile([128, 2], f32, tag="mv")
                        nc.vector.bn_aggr(out=mv, in_=stats)
                        nc.scalar.activation(
                            out=mv[:, 1:2], in_=mv[:, 1:2],
                            func=AF.Sqrt, bias=eps_t[:, :])
                        nc.vector.reciprocal(out=mv[:, 1:2], in_=mv[:, 1:2])
                        nc.vector.tensor_scalar(
                            out=h, in0=h, scalar1=mv[:, 0:1],
                            scalar2=mv[:, 1:2],
                            op0=mybir.AluOpType.subtract,
                            op1=mybir.AluOpType.mult)
                        if g_bc is not None:
                            nc.vector.tensor_mul(out=h, in0=h, in1=g_bc[:, :])
                            nc.vector.tensor_add(out=h, in0=h, in1=b_bc[:, :])
                        if out_dma:
                            nc.sync.dma_start(
                                out=out.rearrange(
                                    "(st p) d -> p st d", p=128)[:, st, :],
                                in_=h)

                # LN1 gamma/beta are folded host-side into W1/b1 (FFN path)
                # and into b2r (residual path); apply only the normalize here
                # so h1T/FFN1 can start sooner.  mha holds y = normalized.
                layer_norm_inplace(None, None, add_in=xh_sb)

                # h1T[d, s] in bf16 (FFN1 moving operand); PSUM evictions
                # split between ACT and DVE
                h1T = bpool.tile([128, 8, TOK], bf16, tag="h1T")
                with tc.tile_pool(name="psT", bufs=1, space="PSUM") as psT:
                    for st in range(8):
                        for dt in range(8):
                            tr_ps = psT.tile([128, 128], f32, tag="tr", bufs=4)
                            nc.tensor.transpose(
                                out=tr_ps,
                                in_=mha[:, st, dt * 128:(dt + 1) * 128],
                                identity=ident_f[:, :])
                            nc.vector.tensor_copy(
                                out=h1T[:, dt, st * 128:(st + 1) * 128],
                                in_=tr_ps)

                # residual stream: h1 = y*g1 + (ln1_b folded into b2r)
                for st in range(8):
                    nc.vector.tensor_mul(out=mha[:, st, :], in0=mha[:, st, :],
                                         in1=g1_bc[:, :])

                # FFN pass 1: aT[Fc] = gelu(W1^T h1 + b1) -> SBUF (bf16)
                aT_all = bpool.tile([128, 32, TOK], bf16, tag="aT")
                with tc.tile_pool(name="ps1", bufs=1, space="PSUM") as ps1:
                    for fc in range(32):
                        w1t = stream.tile([128, 8, 128], bf16, tag="w1t", bufs=6)
                        nc.sync.dma_start(out=w1t, in_=w1[fc])
                        a_ps = ps1.tile([128, TOK], f32, tag="aps", bufs=3)
                        for nh in range(2):
                            for dt in range(8):
                                nc.tensor.matmul(
                                    out=a_ps[:, nh * 512:(nh + 1) * 512],
                                    lhsT=w1t[:, dt, :],
                                    rhs=h1T[:, dt, nh * 512:(nh + 1) * 512],
                                    start=(dt == 0), stop=(dt == 7))
                        nc.scalar.activation(
                            out=aT_all[:, fc, :], in_=a_ps, func=AF.Gelu,
                            bias=b1_sb[:, fc:fc + 1])

                # FFN pass 2: ff = aT^T @ W2 + b2; z2 = h1 + ff (into mha)
                with tc.tile_pool(name="ps2", bufs=1, space="PSUM") as ps2:
                    for sh in range(2):
                        ffps = [ps2.tile([128, D], f32, tag="ff", bufs=4,
                                         name=f"ff_{sh}_{i}")
                                for i in range(4)]
                        for fc in range(32):
                            w2t = stream.tile([128, D], bf16, tag="w2t")
                            nc.scalar.dma_start(
                                out=w2t, in_=w2[fc * 128:(fc + 1) * 128, :])
                            for st2 in range(4):
                                for nh in range(2):
                                    nc.tensor.matmul(
                                        out=ffps[st2][:, nh * 512:(nh + 1) * 512],
                                        lhsT=aT_all[:, fc,
                                                    (sh * 4 + st2) * 128:
                                                    (sh * 4 + st2 + 1) * 128],
                                        rhs=w2t[:, nh * 512:(nh + 1) * 512],
                                        start=(fc == 0), stop=(fc == 31))
                        for st2 in range(4):
                            stg = sh * 4 + st2
                            nc.vector.tensor_add(
                                out=mha[:, stg, :], in0=mha[:, stg, :],
                                in1=ffps[st2][:, :])
                            nc.vector.tensor_add(
                                out=mha[:, stg, :], in0=mha[:, stg, :],
                                in1=ffb2_bc[:, :])

                layer_norm_inplace(g2_bc, b2_bc, out_dma=True)

    nc.compile()
    return nc


def _pack_inputs(x, Wq, bq, Wk, bk, Wv, bv, ln1_g, ln1_b, W1, b1, W2, b2,
                 ln2_g, ln2_b):
    """Build the 8 per-core input maps (host-side, numpy)."""
    import ml_dtypes
    import concourse.mybir as mybir

    f = np.float32
    bf = ml_dtypes.bfloat16
    f8 = mybir.dt.np(mybir.dt.float8e4)

    def pack_w(arr, groups, m):
        # [D, groups*m] -> [groups, 128, 8, m] with [g][p][dt][:] contiguous
        return np.ascontiguousarray(
            arr.reshape(8, 128, groups, m).transpose(2, 1, 0, 3))

    wq_all = (np.ascontiguousarray(
        np.transpose(np.asarray(Wq, f), (1, 0, 2)).reshape(D, D)) * WS
    ).astype(f8)
    wk_all = (np.ascontiguousarray(
        np.transpose(np.asarray(Wk, f), (1, 0, 2)).reshape(D, D)) * WS
    ).astype(f8)
    Wv_ = np.asarray(Wv, f)
    bv_ = np.asarray(bv, f)
    wv_all = np.zeros((D, 4, 260), f)
    bv_all = np.zeros((1, 4, 260), f)
    for quad in range(4):
        for j in range(4):
            h = quad * 4 + j
            wv_all[:, quad, 65 * j:65 * j + 64] = Wv_[h]
            bv_all[0, quad, 65 * j:65 * j + 64] = bv_[h]
            bv_all[0, quad, 65 * j + 64] = 1.0
    wv_p = np.ascontiguousarray(
        ((wv_all * WS).astype(f8)).reshape(8, 128, 4, 260).transpose(2, 1, 0, 3))
    # QK biases pre-scaled so fp8-scaled PSUM + bias stays on one scale
    bqk = np.stack([np.asarray(bq, f).reshape(D) * PS,
                    np.asarray(bk, f).reshape(D) * PS])
    x = np.asarray(x, f)
    W1_ = np.asarray(W1, np.float64)
    g1_ = np.asarray(ln1_g, np.float64)
    bb1_ = np.asarray(ln1_b, np.float64)
    w1_folded = (g1_[:, None] * W1_).astype(f)
    b1_folded = (np.asarray(b1, np.float64) + bb1_ @ W1_).astype(f)
    b2_folded = (np.asarray(b2, np.float64) + bb1_).astype(f)
    common = dict(
        wq=pack_w(wq_all, 8, 128), wk=pack_w(wk_all, 8, 128), wv=wv_p,
        bqk=bqk, bv4=bv_all,
        w1=pack_w(w1_folded.astype(bf), 32, 128), b1=b1_folded,
        w2=np.asarray(W2, f).astype(bf),
        b2r=b2_folded.reshape(1, D),
        ln1g=np.asarray(ln1_g, f),
        ln2g=np.asarray(ln2_g, f), ln2b=np.asarray(ln2_b, f))
    in_maps = []
    for c in range(N_CORES):
        b_, half = c // 2, c % 2
        m = dict(common)
        own = x[b_, half * TOK:(half + 1) * TOK]
        other = x[b_, (1 - half) * TOK:(2 - half) * TOK]
        xTc = (np.ascontiguousarray(
            np.concatenate([own, other], axis=0).T) * XS).astype(f8)
        m["xT"] = np.ascontiguousarray(
            xTc.reshape(8, 128, S).transpose(1, 0, 2))
        m["xh"] = np.ascontiguousarray(own)
        in_maps.append(m)
    return in_maps


def kernel(**inputs):
    from concourse.bass_utils import run_bass_kernel_spmd

    if "nc" not in _BUILD_CACHE:
        _BUILD_CACHE["nc"] = _build()
    nc = _BUILD_CACHE["nc"]
    in_maps = _pack_inputs(**inputs)
    res = run_bass_kernel_spmd(nc, in_maps, core_ids=list(range(N_CORES)))
    out = np.zeros((B, S, D), np.float32)
    for c in range(N_CORES):
        b_, half = c // 2, c % 2
        out[b_, half * TOK:(half + 1) * TOK] = res.results[c]["out"]
    return out
